# revision 1
# baseline (speedup 1.0000x reference)
"""Trainium2 Bass kernel for nn_Conv2d_NN (retrieval_knn).

Reference computation (per batch b):
  xf = x.reshape(B, C, T)                       # T = H*W = 4096, C = 32
  xn = xf / ||xf||_2(channel axis)              # cosine-normalize tokens
  sim = clip(xn^T xn, -1, 1)                    # [T, T]
  vals, idx = top_k(sim, 9)                     # per row, sorted desc
  prime[c,t,k] = vals[t,k] * xf[c, idx[t,k]]
  out[o,t] = sum_{c,k} prime[c,t,k] * w[o,c,k] + bias[o]

Sharding: data-parallel over batch, one batch per NeuronCore (8 cores).

Per-core device algorithm (flash-style fused top-k, sim never hits HBM):
  stage 1: per-token inverse norms via PE transposes + ACT square-accum;
           normalized xn replicated onto all 4 PE row-groups [128, T].
  stage 2: software-pipelined super-blocks (4 row blocks of 128 tokens).
    Per iteration sb, emitted in this order so no engine queue ever has
    an instruction waiting on future work of another engine:
      out(sb-2):    DVE val-scale multiply, PE conv matmuls, ACT bias
                    eviction, DRAM store  (inputs ready since iter sb-1)
      gather(sb-1): gpsimd ap_gather of neighbor columns (idx DMAs from
                    iter sb-1 long complete)
      rows(sb):     PE 4-way row-group-packed fp32 sim matmuls, ACT PSUM
                    eviction, gpsimd diagonal mask, DVE max8/find_index8
      dma(sb):      sync-queue DMAs: idx16 -> gpsimd wrapped index tiles,
                    vals9 -> [32, NI] broadcast row (p-major)
    The DVE top-k scans (2 full passes per row block) are the critical
    engine; everything else hides underneath them.

Gather column order: j = (q*36 + r*9 + k)*16 + pp where the token is
p = pp*8 + q of row block r (q in [0,8), pp in [0,16)) and k is the
neighbor slot.  This is ap_gather's natural wrapped index order, builds
from idx16 [128, 36] with one DMA per 16-partition replica, and keeps
each k-slice of the gathered matrix an affine matmul access pattern
whose walk order is exactly super-block token order.  The vals tile is
broadcast p-major (col = p*36 + rk) and read through a j-order strided
view in the single [32, NI] scale multiply.
"""

import sys

if "/opt/trn_rl_repo" not in sys.path:
    sys.path.insert(0, "/opt/trn_rl_repo")

import numpy as np

B, C, H, W = 8, 32, 64, 64
T = H * W          # 4096
KNN = 9            # neighbors
NCORES = 8
RBS = 128          # row-block size (tokens per block)
NRB = T // RBS     # 32
SUP = 4            # row blocks per super-block
NSUP = NRB // SUP  # 8
SBS = SUP * RBS    # 512 tokens per super-block
CBS = 512          # col-block size (matmul moving dim)
NCB = T // CBS     # 8
O = 32             # conv output channels
RK = SUP * KNN     # 36 (row-block, k) pairs per token-slot group
NI = RBS * RK      # 4608 gathered columns per super-block

_CACHE = {}


def _build_program():
    import concourse.bass as bass
    import concourse.bacc as bacc
    import concourse.mybir as mybir
    from concourse.tile import TileContext, add_dep_helper
    from concourse.masks import make_identity

    f32 = mybir.dt.float32
    i16 = mybir.dt.int16
    u16 = mybir.dt.uint16
    f16 = mybir.dt.float16

    nc = bacc.Bacc("TRN2", target_bir_lowering=False, debug=False,
                   num_devices=NCORES)

    xb = nc.dram_tensor("xb", [C, T], f32, kind="ExternalInput")
    wf = nc.dram_tensor("wf", [KNN * C, O], f16, kind="ExternalInput")
    bias = nc.dram_tensor("bias", [O, 1], f32, kind="ExternalInput")
    out = nc.dram_tensor("out", [O, T], f32, kind="ExternalOutput")

    AF = mybir.ActivationFunctionType
    ALU = mybir.AluOpType

    with TileContext(nc) as tc:
        with (
            tc.tile_pool(name="const", bufs=1) as cpool,
            tc.tile_pool(name="xdata", bufs=1) as xpool,
        ):
            ident128 = cpool.tile([128, 128], f32)
            make_identity(nc, ident128[:])
            ident32 = cpool.tile([32, 32], f32)
            make_identity(nc, ident32[:])
            # rep4[c, m] = 1 iff m % 32 == c: replicates [32, N] onto all
            # four 32-partition row groups via one exact matmul
            rep4 = cpool.tile([C, 128], f32)
            for g in range(4):
                nc.vector.tensor_copy(rep4[:, 32 * g:32 * (g + 1)],
                                      ident32[:])
            # iota4[p, r] = p + r*128 (token id of partition p in row blk r)
            iota4 = cpool.tile([128, SUP], u16)
            nc.gpsimd.iota(iota4[:], pattern=[[RBS, SUP]], base=0,
                           channel_multiplier=1)
            wf_sb = []
            for k in range(KNN):
                wf_k = cpool.tile([C, O], f16, name=f"wf_k{k}")
                nc.sync.dma_start(out=wf_k[:],
                                  in_=wf.ap()[k * C:(k + 1) * C, :])
                wf_sb.append(wf_k)
            ones16 = cpool.tile([1, C], f16)
            nc.gpsimd.memset(ones16[:], 1.0)
            bias_sb = cpool.tile([O, 1], f32)
            nc.sync.dma_start(out=bias_sb[:], in_=bias.ap())

            # raw x [32, T]: gather source (ap_gather channels=32 reads
            # only partitions 0-31) and stage-1 input
            xb_sb = xpool.tile([C, T], f32)
            nc.sync.dma_start(out=xb_sb[:], in_=xb.ap())
            # xn replicated onto all four 32-partition row groups
            xn_rep = xpool.tile([128, T], f32)

            # ---- stage 1: inverse norms, normalized + replicated xn ----
            with (
                tc.tile_pool(name="s1ps", bufs=2, space="PSUM") as s1ps,
                tc.tile_pool(name="s1sb", bufs=3) as s1sb,
            ):
                for blk in range(NRB):
                    cs = slice(blk * RBS, (blk + 1) * RBS)
                    tp = s1ps.tile([RBS, C], f32, tag="tp")
                    nc.tensor.matmul(tp[:], lhsT=xb_sb[:, cs],
                                     rhs=ident32[:], is_transpose=True)
                    xT_blk = s1sb.tile([RBS, C], f32, tag="xT_blk")
                    nc.scalar.activation(xT_blk[:], tp[:], AF.Copy)
                    sq = s1sb.tile([RBS, C], f32, tag="sq")
                    nsq = s1sb.tile([RBS, 1], f32, tag="nsq")
                    nc.scalar.activation(sq[:], xT_blk[:], AF.Square,
                                         accum_out=nsq[:])
                    nrm = s1sb.tile([RBS, 1], f32, tag="nrm")
                    nc.scalar.activation(nrm[:], nsq[:], AF.Sqrt)
                    rinv = s1sb.tile([RBS, 1], f32, tag="rinv")
                    nc.vector.reciprocal(rinv[:], nrm[:])
                    xnT_blk = s1sb.tile([RBS, C], f32, tag="xnT_blk")
                    nc.vector.tensor_scalar_mul(xnT_blk[:], xT_blk[:], rinv[:])
                    # transpose back, then replicate onto all 4 row groups
                    tp2 = s1ps.tile([C, RBS], f32, tag="tp2")
                    nc.tensor.matmul(tp2[:], lhsT=xnT_blk[:],
                                     rhs=ident128[:], is_transpose=True)
                    xn_blk = s1sb.tile([C, RBS], f32, tag="xn_blk")
                    nc.scalar.activation(xn_blk[:], tp2[:], AF.Copy)
                    tp3 = s1ps.tile([128, RBS], f32, tag="tp3")
                    nc.tensor.matmul(tp3[:], lhsT=rep4[:], rhs=xn_blk[:],
                                     start=True, stop=True)
                    nc.scalar.activation(xn_rep[:, cs], tp3[:], AF.Copy)

            # ---- stage 2: fused sim + top-k + gather + conv ----
            tc.strict_bb_all_engine_barrier()
            with (
                tc.tile_pool(name="simps", bufs=4, space="PSUM") as simps,
                tc.tile_pool(name="vps", bufs=2, space="PSUM") as vps,
                tc.tile_pool(name="ops", bufs=1, space="PSUM") as ops,
                tc.tile_pool(name="row", bufs=2) as rowpool,
                tc.tile_pool(name="small", bufs=3) as spool,
                tc.tile_pool(name="big", bufs=2) as bpool,
                tc.tile_pool(name="vbp", bufs=1) as vbpool,
            ):
                tiles = {}

                def stage_row(sb, r):
                    if r == 0:
                        vals9 = spool.tile([RBS, RK], f32, tag="vals9")
                        idx16 = spool.tile([RBS, RK], u16, tag="idx16")
                        tiles[("vals9", sb)] = vals9
                        tiles[("idx16", sb)] = idx16
                        v3 = vals9[:].rearrange("p (r k) -> p r k", r=SUP)
                        i3 = idx16[:].rearrange("p (r k) -> p r k", r=SUP)
                        ms = nc.gpsimd.memset(v3[:, :, 0:1], 1.0)
                        # pin the previous gather ahead of this iteration's
                        # gpsimd preroll: if the scheduler hoists later ops
                        # before the gather, the gather's HWDGE-sem wait
                        # arms after the idx DMA completed and misses the
                        # wake event (~100us timeout poll per super block)
                        if ("gather_inst", sb - 1) in tiles:
                            add_dep_helper(
                                ms.ins, tiles[("gather_inst", sb - 1)].ins,
                                sync=False,
                                reason="gather before next-iter gpsimd ops")
                        nc.gpsimd.tensor_scalar_add(
                            i3[:, :, 0:1],
                            iota4[:].rearrange("p (r one) -> p r one", one=1),
                            sb * SBS)
                    vals9 = tiles[("vals9", sb)]
                    idx16 = tiles[("idx16", sb)]
                    v3 = vals9[:].rearrange("p (r k) -> p r k", r=SUP)
                    i3 = idx16[:].rearrange("p (r k) -> p r k", r=SUP)
                    rb = sb * SUP + r
                    rs = slice(rb * RBS, (rb + 1) * RBS)
                    simrow = rowpool.tile([RBS, T], f32, tag="simrow")
                    # 8 col blocks of 4-way row-group-packed fp32
                    # matmuls, one [128,512] psum bank each
                    for cb in range(NCB):
                        g = cb % 4
                        cs2 = slice(cb * CBS, (cb + 1) * CBS)
                        ps = simps.tile([RBS, CBS], f32, tag="ps", name="ps")
                        nc.tensor.matmul(
                            ps[:],
                            lhsT=xn_rep[32 * g:32 * (g + 1), rs],
                            rhs=xn_rep[32 * g:32 * (g + 1), cs2],
                            tile_position=(32 * g, 0),
                            start=True, stop=True,
                            skip_group_check=True)
                        nc.scalar.activation(simrow[:, cs2], ps[:], AF.Copy)
                    # mask self-similarity to -2
                    nc.gpsimd.affine_select(
                        out=simrow[:, rs], in_=simrow[:, rs],
                        pattern=[[-1, RBS]], channel_multiplier=1, base=0,
                        compare_op=ALU.not_equal, fill=-2.0)
                    nc.vector.max(out=v3[:, r, 1:KNN], in_=simrow[:])
                    nc.vector.max_index(
                        out=i3[:, r, 1:KNN],
                        in_max=v3[:, r, 1:KNN], in_values=simrow[:])

                def stage_dma(sb):
                    idx16 = tiles[("idx16", sb)]
                    vals9 = tiles[("vals9", sb)]
                    # wrapped index tile for ap_gather (2 replicas of 16
                    # partitions for Q7 cores 0 and 1)
                    idxw = spool.tile([32, NI // 16], i16, tag="idxw")
                    for gr in range(2):
                        nc.sync.dma_start(
                            out=idxw[gr * 16:(gr + 1) * 16, :].rearrange(
                                "pp (q rk) -> pp q rk", q=8),
                            in_=idx16[:].bitcast(i16))
                    # vals row, p-major: vrow[0, p*36+rk] = vals9h[p, rk]
                    vals9h = spool.tile([RBS, RK], f16, tag="vals9h")
                    nc.scalar.activation(vals9h[:], vals9[:], AF.Copy)
                    vrow = spool.tile([1, NI], f16, tag="vrow")
                    nc.sync.dma_start(out=vrow[:], in_=vals9h[:])
                    tiles[("idxw", sb)] = idxw
                    tiles[("vrow", sb)] = vrow

                def stage_gather(sb):
                    idxw = tiles[("idxw", sb)]
                    gg = bpool.tile([C, NI], f32, tag="gg")
                    gi = nc.gpsimd.ap_gather(
                        out_ap=gg[:].rearrange("p (n d) -> p n d", d=1),
                        in_ap=xb_sb[:].rearrange("p (n d) -> p n d", d=1),
                        idxs_ap=idxw[:],
                        channels=32, num_elems=T, d=1, num_idxs=NI)
                    tiles[("gg", sb)] = gg
                    tiles[("gather_inst", sb)] = gi

                def stage_out(sb):
                    gg = tiles[("gg", sb)]
                    vrow = tiles[("vrow", sb)]
                    # j-order view of the p-major vals row (contiguous
                    # qrk = q*36+rk inner block, pp stride 288)
                    vrowj = vrow[:].rearrange("one (pp qrk) -> one qrk pp",
                                              pp=16)
                    # broadcast vals to 32 partitions via fp16 ones-matmul,
                    # ACT-evict to SBUF, multiply into gathered columns
                    vb_sb = vbpool.tile([C, NI], f32, tag="vb_sb")
                    pp_t = bpool.tile([C, NI], f16, tag="pp_t")
                    CH = 512
                    for c0 in range(0, NI, CH):
                        c1 = min(c0 + CH, NI)
                        vb_ps = vps.tile([C, CH], f32, tag="vb_ps",
                                         name="vb_ps")
                        nc.tensor.matmul(
                            vb_ps[:, :c1 - c0], lhsT=ones16[:],
                            rhs=vrowj[:, c0 // 16:c1 // 16, :],
                            start=True, stop=True)
                        nc.scalar.activation(vb_sb[:, c0:c1],
                                             vb_ps[:, :c1 - c0], AF.Copy)
                    for c0 in range(0, NI, CH):
                        c1 = min(c0 + CH, NI)
                        nc.vector.tensor_tensor(
                            out=pp_t[:, c0:c1], in0=gg[:, c0:c1],
                            in1=vb_sb[:, c0:c1], op=ALU.mult)
                    out_ps = ops.tile([O, SBS], f32, tag="out_ps")
                    # per-k view, walk (r, pp, q) == super-block token order
                    pview = pp_t[:].rearrange(
                        "c (q r k pp) -> c k r pp q", q=8, r=SUP, k=KNN)
                    for k in range(KNN):
                        nc.tensor.matmul(out_ps[:], lhsT=wf_sb[k][:],
                                         rhs=pview[:, k],
                                         start=(k == 0), stop=(k == KNN - 1))
                    out_sb = spool.tile([O, SBS], f32, tag="out_sb")
                    nc.scalar.activation(out_sb[:], out_ps[:], AF.Identity,
                                         bias=bias_sb[:])
                    nc.scalar.dma_start(
                        out=out.ap()[:, sb * SBS:(sb + 1) * SBS],
                        in_=out_sb[:])

                for sb in range(NSUP):
                    stage_row(sb, 0)
                    if sb >= 1:
                        stage_out(sb - 1)
                    for r in range(1, SUP):
                        stage_row(sb, r)
                    stage_dma(sb)
                    stage_gather(sb)
                stage_out(NSUP - 1)
    nc.compile()
    return nc


def _get_program():
    if "nc" not in _CACHE:
        _CACHE["nc"] = _build_program()
    return _CACHE["nc"]


def _prep_inputs(x, weight, bias):
    xf = np.ascontiguousarray(np.asarray(x, dtype=np.float32).reshape(B, C, T))
    # wf[(k,c), o] = weight[o, c, k]
    wfm = np.ascontiguousarray(
        np.asarray(weight, dtype=np.float32).transpose(2, 1, 0).reshape(
            KNN * C, O).astype(np.float16))
    bp = np.ascontiguousarray(np.asarray(bias, dtype=np.float32).reshape(O, 1))
    return [
        {"xb": np.ascontiguousarray(xf[b]), "wf": wfm, "bias": bp}
        for b in range(B)
    ]


def kernel(x, weight, bias):
    from concourse import bass_utils

    nc = _get_program()
    in_maps = _prep_inputs(x, weight, bias)
    res = bass_utils.run_bass_kernel_spmd(nc, in_maps,
                                          core_ids=list(range(NCORES)))
    out = np.stack([res.results[b]["out"] for b in range(B)])
    return np.ascontiguousarray(out.reshape(B, O, H, W).astype(np.float32))

